# revision 21
# baseline (speedup 1.0000x reference)
"""CIGLoss (segment_reduce) Trainium2 kernel.

Strategy (data-parallel over batch, per the sharding hint):
  - Each of the 8 NeuronCores owns one image and that image's pixel list
    (segments are image-local: seg // 500 == image).
  - Host-side sharding sorts each image's ~500 segments by length
    (ascending) and packs them into a [128 partitions, sum(L_s)] fp8(e4m3)
    grid: slot s holds segments ranked [s*128, (s+1)*128), padded to
    that slot's own width L_s.  Pad entries are 0; their contribution is
    subtracted exactly on device via npad*w^2*|sums|.
  - The value lookup input[b,0,row,col] happens during host packing (this
    toolchain's walrus mis-lowers per-element indirect DMA: one descriptor
    per contiguous dest run, only the run-start offset honored — verified
    by hardware probes).  All heavy reductions run on device:
        sums_s  = accum-reduce(v_s)        slot 0 on ACT, rest on DVE
        contrib_s = ACT Abs activation: sum(|v*w - sums*w^2|) via
                    scale=w, bias=-sums*w^2, accum_out  (== w*sum|v-mean|)
        pad fix: contrib_s -= npad*w^2*|sums_s|
    then a PE ones-matmul partition-reduce so the output is a single
    [1, ncol] row (a [128,1] store shatters into 16 per-SDMA-engine
    slivers whose completion semaphores trickle in over ~5us).
  - Host sums the 8 per-core partials and divides by B.

Schedule notes (from perfetto traces):
  - v DMAs issue on Sync, aux on Scalar, in parallel; slot0 first.
  - A dummy ACT op pulls the ~1.3us ACT_TABLE_LOAD into the DMA lead-in.
  - Slot 0 is the smallest slot and its sum runs on ACT (Copy+accum) so
    the ACT Abs chain — the critical path — starts without a
    cross-engine hop.
  - The last slot's dev pass is split at column XS: ACT takes [0:XS),
    DVE the rest (subtract + abs-reduce), so both engines finish
    together.  Valid because every non-empty segment in the last slot
    has count >= XS (checked on host), so the ACT part sees no pads.
"""

import numpy as np

_NUM_PATHS = 4000
_P = 128  # SBUF partitions


def _build_nc(key):
    import concourse.bacc as bacc
    import concourse.bass as bass
    import concourse.tile as tile
    from concourse import mybir

    Ls, XS = key
    f32 = mybir.dt.float32
    f16 = mybir.dt.float16
    f8 = mybir.dt.float8e4
    Alu = mybir.AluOpType
    Ax = mybir.AxisListType
    Act = mybir.ActivationFunctionType
    ns = len(Ls)
    off = np.concatenate([[0], np.cumsum(Ls)]).astype(int)
    FREE = int(off[-1])
    Lmax = max(Ls)
    ncolA = ns + (1 if XS else 0)  # dev columns (last slot may be split)
    ncol = ncolA + ns              # + one pad-correction column per slot

    nc = bacc.Bacc("TRN2", debug=False)
    v_d = nc.dram_tensor("vP", [_P, FREE], f8, kind="ExternalInput")
    aux_d = nc.dram_tensor("auxP", [_P, 3 * ns], f32, kind="ExternalInput")
    out_d = nc.dram_tensor("out", [ncol, 1], f32, kind="ExternalOutput")

    with tile.TileContext(nc) as tc:
        with (
            tc.tile_pool(name="big", bufs=1) as big,
            tc.tile_pool(name="small", bufs=1) as small,
            tc.tile_pool(name="psum", bufs=1, space="PSUM") as psum,
        ):
            v_t = big.tile([_P, FREE], f8)
            aux_t = small.tile([_P, 3 * ns], f32)
            nc.sync.dma_start(out=v_t[:, : int(off[1])],
                              in_=v_d[:, : int(off[1])])
            for s in range(1, ns):
                a, b = int(off[s]), int(off[s + 1])
                eng = nc.scalar if s == 1 else nc.sync
                eng.dma_start(out=v_t[:, a:b], in_=v_d[:, a:b])
            nc.gpsimd.dma_start(out=aux_t[:], in_=aux_d[:, :])
            w_v = aux_t[:, 0:ns]
            nww_v = aux_t[:, ns:2 * ns]        # -w^2
            npww_v = aux_t[:, 2 * ns:3 * ns]   # npad*w^2

            ones_t = small.tile([_P, 1], f32)
            nc.vector.memset(ones_t[:], 1.0)
            # trigger the ACT table-set load during the DMA lead-in
            warm = small.tile([_P, 1], f32)
            warmacc = small.tile([_P, 1], f32)
            nc.vector.memset(warm[:], 0.0)
            nc.scalar.activation(
                out=warm[:], in_=warm[:], func=Act.Abs,
                bias=0.0, scale=1.0, accum_out=warmacc[:],
            )

            junk = big.tile([_P, Lmax], f16)      # DVE fused-op dump
            scr = big.tile([_P, Lmax], f16)       # ACT dump (separate:
            sums = small.tile([_P, ns], f32)      # no cross-engine WAW)
            nmw = small.tile([_P, ns], f32)       # -sums*w^2 (ACT bias)
            # comb columns: [0:ncolA) scaled dev sums (ACT accums + devB),
            # [ncolA:) pad-correction terms; host subtracts the two groups
            comb = small.tile([_P, ncol], f32)

            def act_dev(s, hi=None):
                # contrib_s = sum |v*w + (-sums*w^2)| = w * sum|v - mean|
                a = int(off[s])
                b = int(off[s + 1]) if hi is None else a + hi
                nc.scalar.activation(
                    out=scr[:, : b - a], in_=v_t[:, a:b], func=Act.Abs,
                    bias=nmw[:, s:s + 1], scale=w_v[:, s:s + 1],
                    accum_out=comb[:, s:s + 1],
                )

            nc.vector.tensor_scalar(
                out=junk[:, : Ls[0]], in0=v_t[:, : int(off[1])],
                scalar1=1.0, scalar2=None, op0=Alu.mult,
                op1=Alu.add, accum_out=sums[:, 0:1],
            )
            nc.vector.tensor_tensor(
                out=nmw[:, 0:1], in0=sums[:, 0:1], in1=nww_v[:, 0:1],
                op=Alu.mult,
            )
            act_dev(0)
            for s in range(1, ns):
                a, b = int(off[s]), int(off[s + 1])
                nc.vector.tensor_scalar(
                    out=junk[:, : Ls[s]], in0=v_t[:, a:b],
                    scalar1=1.0, scalar2=None, op0=Alu.mult,
                    op1=Alu.add, accum_out=sums[:, s:s + 1],
                )
                nc.vector.tensor_tensor(
                    out=nmw[:, s:s + 1], in0=sums[:, s:s + 1],
                    in1=nww_v[:, s:s + 1], op=Alu.mult,
                )
                act_dev(s, hi=XS if s == ns - 1 else None)

            sl = ns - 1
            if XS:
                # DVE takes columns [XS:] of the last slot:
                # junk = v*w + (-sums*w^2) = w*(v - mean), then abs-reduce
                a3, b3 = int(off[sl]) + XS, int(off[ns])
                nc.vector.tensor_scalar(
                    out=junk[:, : b3 - a3], in0=v_t[:, a3:b3],
                    scalar1=w_v[:, sl:sl + 1], scalar2=nmw[:, sl:sl + 1],
                    op0=Alu.mult, op1=Alu.add,
                )
                nc.vector.tensor_reduce(
                    out=comb[:, ns:ns + 1], in_=junk[:, : b3 - a3],
                    axis=Ax.X, op=Alu.add, apply_absolute_value=True,
                )

            # pad correction: corr = npad*w^2*|sums|; all pads of the
            # split slot sit in the DVE region (count >= XS), so the
            # ACT part (contrib col sl) needs no correction
            neg_s = small.tile([_P, ns], f32)
            nc.vector.tensor_scalar(
                out=neg_s[:], in0=sums[:], scalar1=-1.0, scalar2=None,
                op0=Alu.mult,
            )
            nc.vector.tensor_tensor(
                out=neg_s[:], in0=neg_s[:], in1=sums[:], op=Alu.max
            )
            nc.vector.tensor_tensor(
                out=comb[:, ncolA:], in0=neg_s[:], in1=npww_v, op=Alu.mult
            )

            # partition-reduce on the PE -> [1, ncol] single-desc out
            part_p = psum.tile([ncol, 1], f32)
            nc.tensor.matmul(
                part_p[:], comb[:], ones_t[:], start=True, stop=True
            )
            part = small.tile([ncol, 1], f32)
            nc.vector.tensor_copy(out=part[:], in_=part_p[:])
            nc.scalar.dma_start(out=out_d[:, :], in_=part[:])
    nc.finalize()
    return nc


_CACHE = {}


def _get_nc(key):
    if key not in _CACHE:
        _CACHE[key] = _build_nc(key)
    return _CACHE[key]


def _pack(input, rows, cols, seg_ids, num_paths):
    """Host-side sharding: one image per core; segments sorted by length
    (ascending) and packed into a [ncore, 128, sum(L_s)] fp16 slot grid."""
    B, C, H, W = input.shape
    ppi = num_paths // B
    npix = rows.shape[0]

    bnd = np.searchsorted(seg_ids, np.arange(num_paths + 1)).astype(np.int64)
    seg_lens = np.diff(bnd)                       # [num_paths]
    ns = (ppi + _P - 1) // _P
    lens_c = seg_lens.reshape(B, ppi)
    order = np.argsort(lens_c, axis=1, kind="stable")    # [B, ppi] asc
    rank = np.empty_like(order)
    np.put_along_axis(rank, order, np.arange(ppi)[None, :].repeat(B, 0), 1)

    sorted_lens = np.take_along_axis(lens_c, order, axis=1)
    Ls = []
    for k in range(ns):
        blk = sorted_lens[:, k * _P:(k + 1) * _P]
        m = int(blk.max()) if blk.size else 1
        Ls.append(max(32, -(-m // 32) * 32))
    off = np.concatenate([[0], np.cumsum(Ls)]).astype(np.int64)
    FREE = int(off[-1])

    # ACT/DVE split point for the last slot: balance the two engines,
    # but never exceed the slot's min nonzero length (the ACT region
    # must be pad-free for the correction algebra to hold)
    lb = sorted_lens[:, (ns - 1) * _P:]
    nzmin = int(lb[lb > 0].min()) if (lb > 0).any() else 0
    XS = min(704, nzmin // 32 * 32)
    if XS < 64 or Ls[-1] - XS < 64:
        XS = 0

    s_all = np.arange(num_paths)
    core = s_all // ppi
    r = rank[core, s_all % ppi]                  # sorted rank within image
    slot = r // _P
    part = r % _P
    base = (core * _P + part) * np.int64(FREE) + off[slot]
    dest = np.repeat(base, seg_lens) + (
        np.arange(npix, dtype=np.int64) - np.repeat(bnd[:-1], seg_lens)
    )
    core_of = np.repeat(core, seg_lens)
    import ml_dtypes
    v_p = np.zeros(B * _P * FREE, ml_dtypes.float8_e4m3)
    v_p[dest] = input[core_of, 0, rows, cols].astype(ml_dtypes.float8_e4m3)

    # per-(core, partition, slot) counts -> w, -w^2, npad*w^2  (f32 aux)
    cnt = np.zeros((B, _P, ns), np.float32)
    rr = np.arange(ppi)
    for c in range(B):
        cnt[c, rr % _P, rr // _P] = sorted_lens[c]
    w = (1.0 / np.maximum(cnt, 1.0)).astype(np.float32)
    npad = np.asarray(Ls, np.float32)[None, None, :] - cnt
    aux = np.concatenate([w, -w * w, npad * w * w], axis=2).astype(np.float32)
    return v_p.reshape(B, _P, FREE), aux, tuple(Ls), int(XS)


def kernel(input, rows, cols, seg_ids, _trace=False, _num_paths=_NUM_PATHS):
    from concourse.bass_utils import run_bass_kernel_spmd

    input = np.ascontiguousarray(np.asarray(input, np.float32))
    rows = np.ascontiguousarray(np.asarray(rows, np.int32))
    cols = np.ascontiguousarray(np.asarray(cols, np.int32))
    seg_ids = np.ascontiguousarray(np.asarray(seg_ids, np.int32))
    B, C, H, W = input.shape

    v_p, aux, Ls, XS = _pack(input, rows, cols, seg_ids, _num_paths)
    nc = _get_nc((Ls, XS))
    in_maps = [{"vP": v_p[i], "auxP": aux[i]} for i in range(B)]
    res = run_bass_kernel_spmd(nc, in_maps, core_ids=list(range(B)), trace=_trace)
    ncolA = len(Ls) + (1 if XS else 0)
    total = sum(
        float(r["out"][:ncolA].sum()) - float(r["out"][ncolA:].sum())
        for r in res.results
    )
    out = np.float32(total / B)
    if _trace:
        return out, res
    return out


# revision 22
# speedup vs baseline: 1.0866x; 1.0866x over previous
"""CIGLoss (segment_reduce) Trainium2 kernel.

Strategy (data-parallel over batch, per the sharding hint):
  - Each of the 8 NeuronCores owns one image and that image's pixel list
    (segments are image-local: seg // 500 == image).
  - Host-side sharding sorts each image's ~500 segments by length
    (ascending) and packs them into a [128 partitions, sum(L_s)] fp8(e4m3)
    grid: slot s holds segments ranked [s*128, (s+1)*128), padded to
    that slot's own width L_s.  Pad entries are 0; their contribution is
    subtracted exactly on device via npad*w^2*|sums|.
  - The value lookup input[b,0,row,col] happens during host packing (this
    toolchain's walrus mis-lowers per-element indirect DMA: one descriptor
    per contiguous dest run, only the run-start offset honored — verified
    by hardware probes).  All heavy reductions run on device:
        sums_s  = accum-reduce(v_s)        slot 0 on ACT, rest on DVE
        contrib_s = ACT Abs activation: sum(|v*w - sums*w^2|) via
                    scale=w, bias=-sums*w^2, accum_out  (== w*sum|v-mean|)
        pad fix: contrib_s -= npad*w^2*|sums_s|
    then a PE ones-matmul partition-reduce so the output is a single
    [1, ncol] row (a [128,1] store shatters into 16 per-SDMA-engine
    slivers whose completion semaphores trickle in over ~5us).
  - Host sums the 8 per-core partials and divides by B.

Schedule notes (from perfetto traces):
  - v DMAs issue on Sync, aux on Scalar, in parallel; slot0 first.
  - A dummy ACT op pulls the ~1.3us ACT_TABLE_LOAD into the DMA lead-in.
  - Slot 0 is the smallest slot and its sum runs on ACT (Copy+accum) so
    the ACT Abs chain — the critical path — starts without a
    cross-engine hop.
  - The last slot's dev pass is split at column XS: ACT takes [0:XS),
    DVE the rest (subtract + abs-reduce), so both engines finish
    together.  Valid because every non-empty segment in the last slot
    has count >= XS (checked on host), so the ACT part sees no pads.
"""

import numpy as np

_NUM_PATHS = 4000
_P = 128  # SBUF partitions


def _build_nc(key):
    import concourse.bacc as bacc
    import concourse.bass as bass
    import concourse.tile as tile
    from concourse import mybir

    Ls, XS = key
    f32 = mybir.dt.float32
    f16 = mybir.dt.float16
    f8 = mybir.dt.float8e4
    Alu = mybir.AluOpType
    Ax = mybir.AxisListType
    Act = mybir.ActivationFunctionType
    ns = len(Ls)
    off = np.concatenate([[0], np.cumsum(Ls)]).astype(int)
    FREE = int(off[-1])
    Lmax = max(Ls)
    ncolA = ns + (1 if XS else 0)  # dev columns (last slot may be split)
    ncol = ncolA + ns              # + one pad-correction column per slot

    nc = bacc.Bacc("TRN2", debug=False)
    v_d = nc.dram_tensor("vP", [_P, FREE], f8, kind="ExternalInput")
    aux_d = nc.dram_tensor("auxP", [_P, 3 * ns], f32, kind="ExternalInput")
    out_d = nc.dram_tensor("out", [ncol, 1], f32, kind="ExternalOutput")

    with tile.TileContext(nc) as tc:
        with (
            tc.tile_pool(name="big", bufs=1) as big,
            tc.tile_pool(name="small", bufs=1) as small,
            tc.tile_pool(name="psum", bufs=1, space="PSUM") as psum,
        ):
            v_t = big.tile([_P, FREE], f8)
            aux_t = small.tile([_P, 3 * ns], f32)
            nc.sync.dma_start(out=v_t[:, : int(off[1])],
                              in_=v_d[:, : int(off[1])])
            for s in range(1, ns):
                a, b = int(off[s]), int(off[s + 1])
                eng = nc.scalar if s == 1 else nc.sync
                eng.dma_start(out=v_t[:, a:b], in_=v_d[:, a:b])
            nc.gpsimd.dma_start(out=aux_t[:], in_=aux_d[:, :])
            w_v = aux_t[:, 0:ns]
            nww_v = aux_t[:, ns:2 * ns]        # -w^2
            npww_v = aux_t[:, 2 * ns:3 * ns]   # npad*w^2

            ones_t = small.tile([_P, 1], f32)
            nc.vector.memset(ones_t[:], 1.0)
            # trigger the ACT table-set load during the DMA lead-in
            warm = small.tile([_P, 1], f32)
            warmacc = small.tile([_P, 1], f32)
            nc.vector.memset(warm[:], 0.0)
            nc.scalar.activation(
                out=warm[:], in_=warm[:], func=Act.Abs,
                bias=0.0, scale=1.0, accum_out=warmacc[:],
            )

            junk = big.tile([_P, Lmax], f16)      # DVE fused-op dump
            scr = big.tile([_P, Lmax], f16)       # ACT dump (separate:
            sums = small.tile([_P, ns], f32)      # no cross-engine WAW)
            nmw = small.tile([_P, ns], f32)       # -sums*w^2 (ACT bias)
            # comb columns: [0:ncolA) scaled dev sums (ACT accums + devB),
            # [ncolA:) pad-correction terms; host subtracts the two groups
            comb = small.tile([_P, ncol], f32)

            def act_dev(s, hi=None):
                # contrib_s = sum |v*w + (-sums*w^2)| = w * sum|v - mean|
                a = int(off[s])
                b = int(off[s + 1]) if hi is None else a + hi
                nc.scalar.activation(
                    out=scr[:, : b - a], in_=v_t[:, a:b], func=Act.Abs,
                    bias=nmw[:, s:s + 1], scale=w_v[:, s:s + 1],
                    accum_out=comb[:, s:s + 1],
                )

            nc.vector.tensor_scalar(
                out=junk[:, : Ls[0]], in0=v_t[:, : int(off[1])],
                scalar1=1.0, scalar2=None, op0=Alu.mult,
                op1=Alu.add, accum_out=sums[:, 0:1],
            )
            nc.vector.tensor_tensor(
                out=nmw[:, 0:1], in0=sums[:, 0:1], in1=nww_v[:, 0:1],
                op=Alu.mult,
            )
            act_dev(0)
            for s in range(1, ns):
                a, b = int(off[s]), int(off[s + 1])
                nc.vector.tensor_scalar(
                    out=junk[:, : Ls[s]], in0=v_t[:, a:b],
                    scalar1=1.0, scalar2=None, op0=Alu.mult,
                    op1=Alu.add, accum_out=sums[:, s:s + 1],
                )
                nc.vector.tensor_tensor(
                    out=nmw[:, s:s + 1], in0=sums[:, s:s + 1],
                    in1=nww_v[:, s:s + 1], op=Alu.mult,
                )
                act_dev(s, hi=XS if s == ns - 1 else None)

            sl = ns - 1
            if XS:
                # DVE takes columns [XS:] of the last slot:
                # junk = v*w + (-sums*w^2) = w*(v - mean), then abs-reduce
                a3, b3 = int(off[sl]) + XS, int(off[ns])
                nc.vector.tensor_scalar(
                    out=junk[:, : b3 - a3], in0=v_t[:, a3:b3],
                    scalar1=w_v[:, sl:sl + 1], scalar2=nmw[:, sl:sl + 1],
                    op0=Alu.mult, op1=Alu.add,
                )
                nc.vector.tensor_reduce(
                    out=comb[:, ns:ns + 1], in_=junk[:, : b3 - a3],
                    axis=Ax.X, op=Alu.add, apply_absolute_value=True,
                )

            # pad correction: corr = npad*w^2*|sums|; all pads of the
            # split slot sit in the DVE region (count >= XS), so the
            # ACT part (contrib col sl) needs no correction
            neg_s = small.tile([_P, ns], f32)
            nc.vector.tensor_scalar(
                out=neg_s[:], in0=sums[:], scalar1=-1.0, scalar2=None,
                op0=Alu.mult,
            )
            nc.vector.tensor_tensor(
                out=neg_s[:], in0=neg_s[:], in1=sums[:], op=Alu.max
            )
            nc.vector.tensor_tensor(
                out=comb[:, ncolA:], in0=neg_s[:], in1=npww_v, op=Alu.mult
            )

            # partition-reduce on the PE -> [1, ncol] single-desc out
            part_p = psum.tile([ncol, 1], f32)
            nc.tensor.matmul(
                part_p[:], comb[:], ones_t[:], start=True, stop=True
            )
            part = small.tile([ncol, 1], f32)
            nc.vector.tensor_copy(out=part[:], in_=part_p[:])
            nc.sync.dma_start(out=out_d[:, :], in_=part[:])
    nc.finalize()
    return nc


_CACHE = {}


def _get_nc(key):
    if key not in _CACHE:
        _CACHE[key] = _build_nc(key)
    return _CACHE[key]


def _pack(input, rows, cols, seg_ids, num_paths):
    """Host-side sharding: one image per core; segments sorted by length
    (ascending) and packed into a [ncore, 128, sum(L_s)] fp16 slot grid."""
    B, C, H, W = input.shape
    ppi = num_paths // B
    npix = rows.shape[0]

    bnd = np.searchsorted(seg_ids, np.arange(num_paths + 1)).astype(np.int64)
    seg_lens = np.diff(bnd)                       # [num_paths]
    ns = (ppi + _P - 1) // _P
    lens_c = seg_lens.reshape(B, ppi)
    order = np.argsort(lens_c, axis=1, kind="stable")    # [B, ppi] asc
    rank = np.empty_like(order)
    np.put_along_axis(rank, order, np.arange(ppi)[None, :].repeat(B, 0), 1)

    sorted_lens = np.take_along_axis(lens_c, order, axis=1)
    Ls = []
    for k in range(ns):
        blk = sorted_lens[:, k * _P:(k + 1) * _P]
        m = int(blk.max()) if blk.size else 1
        Ls.append(max(32, -(-m // 32) * 32))
    off = np.concatenate([[0], np.cumsum(Ls)]).astype(np.int64)
    FREE = int(off[-1])

    # ACT/DVE split point for the last slot: balance the two engines,
    # but never exceed the slot's min nonzero length (the ACT region
    # must be pad-free for the correction algebra to hold)
    lb = sorted_lens[:, (ns - 1) * _P:]
    nzmin = int(lb[lb > 0].min()) if (lb > 0).any() else 0
    XS = min(704, nzmin // 32 * 32)
    if XS < 64 or Ls[-1] - XS < 64:
        XS = 0

    s_all = np.arange(num_paths)
    core = s_all // ppi
    r = rank[core, s_all % ppi]                  # sorted rank within image
    slot = r // _P
    part = r % _P
    base = (core * _P + part) * np.int64(FREE) + off[slot]
    dest = np.repeat(base, seg_lens) + (
        np.arange(npix, dtype=np.int64) - np.repeat(bnd[:-1], seg_lens)
    )
    core_of = np.repeat(core, seg_lens)
    import ml_dtypes
    v_p = np.zeros(B * _P * FREE, ml_dtypes.float8_e4m3)
    v_p[dest] = input[core_of, 0, rows, cols].astype(ml_dtypes.float8_e4m3)

    # per-(core, partition, slot) counts -> w, -w^2, npad*w^2  (f32 aux)
    cnt = np.zeros((B, _P, ns), np.float32)
    rr = np.arange(ppi)
    for c in range(B):
        cnt[c, rr % _P, rr // _P] = sorted_lens[c]
    w = (1.0 / np.maximum(cnt, 1.0)).astype(np.float32)
    npad = np.asarray(Ls, np.float32)[None, None, :] - cnt
    aux = np.concatenate([w, -w * w, npad * w * w], axis=2).astype(np.float32)
    return v_p.reshape(B, _P, FREE), aux, tuple(Ls), int(XS)


def kernel(input, rows, cols, seg_ids, _trace=False, _num_paths=_NUM_PATHS):
    from concourse.bass_utils import run_bass_kernel_spmd

    input = np.ascontiguousarray(np.asarray(input, np.float32))
    rows = np.ascontiguousarray(np.asarray(rows, np.int32))
    cols = np.ascontiguousarray(np.asarray(cols, np.int32))
    seg_ids = np.ascontiguousarray(np.asarray(seg_ids, np.int32))
    B, C, H, W = input.shape

    v_p, aux, Ls, XS = _pack(input, rows, cols, seg_ids, _num_paths)
    nc = _get_nc((Ls, XS))
    in_maps = [{"vP": v_p[i], "auxP": aux[i]} for i in range(B)]
    res = run_bass_kernel_spmd(nc, in_maps, core_ids=list(range(B)), trace=_trace)
    ncolA = len(Ls) + (1 if XS else 0)
    total = sum(
        float(r["out"][:ncolA].sum()) - float(r["out"][ncolA:].sum())
        for r in res.results
    )
    out = np.float32(total / B)
    if _trace:
        return out, res
    return out
